# revision 15
# baseline (speedup 1.0000x reference)
"""Multi-head causal attention (B=4, S=2048, E=1024, H=16, D=64) on 8 TRN2
NeuronCores. Head-sharded tensor parallelism: each core computes 2 heads for
all batches plus its 128-row slice of the output projection; the host sums
the 8 partial outputs.

v2: host-side x transpose (no transpose DMAs), ACT engine reserved for exp
(out-DMA and psum copies moved to SP/DVE/Pool), st-sliced x loads, per-qi
batched output DMA, 65-col V+ones layout.

Self-contained: hardcodes shapes/sharding; only depends on /opt/trn_rl_repo.
"""
import sys
from contextlib import ExitStack

sys.path.insert(0, "/opt/trn_rl_repo")

import numpy as np
import ml_dtypes

import concourse.bass as bass  # noqa: F401  (registers engine types)
import concourse.bacc as bacc
import concourse.mybir as mybir
import concourse.tile as tile
from concourse.bass_utils import run_bass_kernel_spmd
from concourse.masks import make_upper_triangular

BF16 = mybir.dt.bfloat16
F32 = mybir.dt.float32
NBF = ml_dtypes.bfloat16

B, S, E, H, D = 4, 2048, 1024, 16, 64
NCORES = 8
HPC = 2          # heads per core
D2 = HPC * D     # 128
QT_ = 512        # q tile width
KC_ = 128        # k chunk width
VW = 65          # V columns + 1 ones column (softmax denominator)
EXP_FN = mybir.ActivationFunctionType.Exp
MULT = mybir.AluOpType.mult


def build_program(b=B, s=S, e=E, repeat=1, phases=("tr", "qkv", "attn", "oproj")):
    """One SPMD program, identical on all 8 cores."""
    phases = set(phases)
    assert s % QT_ == 0 and e % 128 == 0
    EC = e // 128            # contraction chunks
    NQ = s // QT_            # q tiles per batch
    NK = s // KC_            # k chunks per batch

    nc = bacc.Bacc("TRN2", target_bir_lowering=False, debug=False,
                   num_devices=NCORES)
    with tile.TileContext(nc) as tc, ExitStack() as ctx:
        with tc.tile_pool(name="dram", bufs=1, space="DRAM") as dram:
            # x, blocked host-side as [p, b, st, ec, t] so every DMA is one
            # 8KB contiguous run per partition (128 descriptors per slice)
            xb_d = dram.tile([128, b, s // QT_, EC, QT_], BF16,
                             kind="ExternalInput", name="xb", uniquify=False)
            wq_d = dram.tile([e, D2], BF16, kind="ExternalInput",
                             name="wq", uniquify=False)
            wk_d = dram.tile([e, D2], BF16, kind="ExternalInput",
                             name="wk", uniquify=False)
            wv_d = dram.tile([e, D2], BF16, kind="ExternalInput",
                             name="wv", uniquify=False)
            wo_d = dram.tile([D2, e], BF16, kind="ExternalInput",
                             name="wo", uniquify=False)
            bq_d = dram.tile([D2, 1], F32, kind="ExternalInput",
                             name="bq", uniquify=False)
            bk_d = dram.tile([D2, 1], F32, kind="ExternalInput",
                             name="bk", uniquify=False)
            # output, blocked [p, r, e]: host maps row (r, p) -> r*128+p
            out_d = dram.tile([128, b * s // 128, e], F32,
                              kind="ExternalOutput", name="out", uniquify=False)

            const = ctx.enter_context(tc.tile_pool(name="const", bufs=1))
            wpool = ctx.enter_context(tc.tile_pool(name="wpool", bufs=1))
            xtp = ctx.enter_context(tc.tile_pool(name="xtp", bufs=3))
            qkp = ctx.enter_context(tc.tile_pool(name="qkp", bufs=2))
            vp = ctx.enter_context(tc.tile_pool(name="vp", bufs=2))
            etp = ctx.enter_context(tc.tile_pool(name="etp", bufs=6))
            rp = ctx.enter_context(tc.tile_pool(name="rp", bufs=2))
            orp = ctx.enter_context(tc.tile_pool(name="orp", bufs=2))
            pp = ctx.enter_context(tc.tile_pool(name="pp", bufs=1, space="PSUM"))

            # constants
            tri2 = const.tile([128, 2, 128], BF16)
            make_upper_triangular(nc, tri2[:, 0, :], val=1.0, diag=True)
            nc.vector.tensor_copy(tri2[:, 1, :], tri2[:, 0, :])
            bq_sb = const.tile([D2, 1], F32)
            nc.sync.dma_start(out=bq_sb[:], in_=bq_d[:])
            bk_sb = const.tile([D2, 1], F32)
            nc.sync.dma_start(out=bk_sb[:], in_=bk_d[:])

            # weights, chunked along contraction dim
            wq_sb = wpool.tile([128, EC, D2], BF16)
            nc.sync.dma_start(out=wq_sb[:], in_=wq_d.rearrange("(c p) d -> p c d", p=128))
            wk_sb = wpool.tile([128, EC, D2], BF16)
            nc.sync.dma_start(out=wk_sb[:], in_=wk_d.rearrange("(c p) d -> p c d", p=128))
            wv_sb = wpool.tile([128, EC, D2], BF16)
            nc.sync.dma_start(out=wv_sb[:], in_=wv_d.rearrange("(c p) d -> p c d", p=128))
            wo_sb = wpool.tile([D2, e], BF16)
            nc.sync.dma_start(out=wo_sb[:], in_=wo_d[:])

            GPK = QT_ // KC_   # k-chunks per q-tile

            def body(_iv=None):
                bt = {}     # per-batch live tiles

                def issue_tr(bi):
                    xt = xtp.tile([128, NQ, EC, QT_], BF16, name="xt")
                    bt[bi] = {"xt": xt}
                    if "tr" in phases:
                        # one DMA per q-tile-slice so the first proj chunks
                        # unblock before the whole batch has landed
                        for st in range(NQ):
                            nc.sync.dma_start(out=xt[:, st], in_=xb_d[:, bi, st])
                    elif "qkv" in phases:
                        nc.gpsimd.memset(xt[:], 0.125)

                def make_proj_chunks(bi):
                    """st-major chunk closures: [Q(st), K(st), V(4st..4st+3)]"""
                    st_ = bt[bi]
                    st_["qt"] = qkp.tile([D2, s], BF16, name="qt")
                    st_["kt"] = qkp.tile([D2, s], BF16, name="kt")
                    st_["vv"] = vp.tile([128, NK, 2, VW], BF16, name="vv")
                    # ones column for the softmax denominator, once per batch
                    nc.gpsimd.memset(st_["vv"][:, :, :, 64:VW], 1.0)
                    if "qkv" not in phases:
                        if "attn" in phases:
                            nc.gpsimd.memset(st_["qt"][:], 0.25)
                            nc.gpsimd.memset(st_["kt"][:], 0.25)
                            nc.gpsimd.memset(st_["vv"][:, :, :, 0:64], 0.25)
                        return []
                    xt = st_["xt"]

                    def qk_chunk(st, which):
                        def go():
                            w_sb, b_sb, dst = ((wq_sb, bq_sb, st_["qt"])
                                               if which == "q" else
                                               (wk_sb, bk_sb, st_["kt"]))
                            cs = slice(st * 512, (st + 1) * 512)
                            psq = pp.tile([128, 512], F32, name="psq",
                                          tag="proj", bufs=2)
                            for ec in range(EC):
                                nc.tensor.matmul(psq[:], w_sb[:, ec, :],
                                                 xt[:, st, ec, :],
                                                 start=(ec == 0), stop=(ec == EC - 1))
                            nc.vector.tensor_scalar_add(dst[:, cs], psq[:], b_sb[:])
                        return go

                    def v_chunk(sc):
                        def go():
                            vv = st_["vv"]
                            psv = pp.tile([128, 128], F32, name="psv",
                                          tag="proj", bufs=2)
                            off = (sc % 4) * 128
                            for ec in range(EC):
                                nc.tensor.matmul(psv[:],
                                                 xt[:, sc // 4, ec, off:off + 128],
                                                 wv_sb[:, ec, :],
                                                 start=(ec == 0), stop=(ec == EC - 1))
                            nc.vector.tensor_copy(
                                vv[:, sc, :, 0:64],
                                psv.rearrange("p (h d) -> p h d", h=2))
                        return go

                    chunks = []
                    for st in range(NQ):
                        chunks.append(qk_chunk(st, "q"))
                        chunks.append(qk_chunk(st, "k"))
                        for sc in range(4 * st, 4 * st + 4):
                            chunks.append(v_chunk(sc))
                    return chunks

                def issue_attn(bi, chunks):
                    """attention for batch bi with proj chunks interleaved"""
                    st_ = bt[bi]
                    qt, kt, vv = st_["qt"], st_["kt"], st_["vv"]
                    ot = qkp.tile([D2, s], BF16, name="ot")
                    row0 = bi * s
                    if "attn" not in phases:
                        for c in chunks:
                            c()
                        if "oproj" in phases:
                            nc.gpsimd.memset(ot[:], 0.25)
                            for qi in range(NQ):
                                issue_oproj(bi, qi, ot)
                        return
                    TK = sum((qi + 1) * GPK for qi in range(NQ))
                    kci = 0
                    issued = 0
                    for qi in range(NQ):
                        pso = [pp.tile([128, 512], F32, name=f"pso{h}",
                                       tag=f"pso{h}", bufs=1) for h in range(HPC)]
                        nkc = (qi + 1) * GPK
                        pend = None
                        for kc in range(nkc):
                            dj = kc - qi * GPK
                            qoff = KC_ * dj if dj >= 0 else 0
                            n = 512 - qoff
                            # scores pair + single exp over both heads
                            pssp = pp.tile([128, 2, 512], F32, name="pssp",
                                           tag="pss", bufs=2)
                            for h in range(HPC):
                                hs = slice(h * 64, (h + 1) * 64)
                                nc.tensor.matmul(
                                    pssp[:, h, 0:n],
                                    kt[hs, kc * 128:(kc + 1) * 128],
                                    qt[hs, qi * 512 + qoff:(qi + 1) * 512],
                                    start=True, stop=True)
                            et = etp.tile([128, 2, 512], BF16, name="et")
                            nc.scalar.activation(et[:, :, 0:n], pssp[:, :, 0:n],
                                                 EXP_FN, scale=0.125)
                            if dj >= 0:
                                nc.gpsimd.tensor_tensor(
                                    et[:, :, 0:128], et[:, :, 0:128],
                                    tri2[:], MULT)
                            # a proj chunk of the neighbour batch, to keep PE fed
                            while chunks and issued < (kci + 1) * len(chunks) // TK:
                                chunks[issued]()
                                issued += 1
                            # AV of the previous k-chunk (distance-1 issue)
                            if pend is not None:
                                flush_av(pso, vv, pend, nkc)
                            pend = (kc, et, qoff, n)
                            kci += 1
                        flush_av(pso, vv, pend, nkc)
                        # normalize + write ot, then the output projection rows
                        for h in range(HPC):
                            r1 = rp.tile([1, 512], F32, name="r1")
                            nc.vector.reciprocal(r1[:], pso[h][64:65, :])
                            rb = rp.tile([64, 512], F32, name="rb")
                            nc.gpsimd.partition_broadcast(rb[:], r1[:])
                            nc.vector.tensor_tensor(
                                ot[h * 64:(h + 1) * 64, qi * 512:(qi + 1) * 512],
                                pso[h][0:64, :], rb[:], MULT)
                        if "oproj" in phases:
                            issue_oproj(bi, qi, ot)
                    while issued < len(chunks):
                        chunks[issued]()
                        issued += 1

                def flush_av(pso, vv, pend, nkc):
                    kc, et, qoff, n = pend
                    for h in range(HPC):
                        nc.tensor.matmul(
                            pso[h][0:VW, qoff:512], vv[:, kc, h, :], et[:, h, 0:n],
                            start=(kc == 0), stop=(kc == nkc - 1),
                            skip_group_check=True)

                def issue_oproj(bi, qi, ot):
                    orow = orp.tile([128, 4, e], F32, name="orow")
                    for j in range(4):
                        sc = 4 * qi + j
                        for eh in range(e // 512):
                            psf = pp.tile([128, 512], F32, name="psf",
                                          tag="proj", bufs=2)
                            nc.tensor.matmul(psf[:], ot[:, sc * 128:(sc + 1) * 128],
                                             wo_sb[:, eh * 512:(eh + 1) * 512],
                                             start=True, stop=True)
                            dst = orow[:, j, eh * 512:(eh + 1) * 512]
                            # GPSIMD can't read PSUM on HW; split copies
                            # between DVE and ACT to balance engine load
                            if eh == 0:
                                nc.vector.tensor_copy(dst, psf[:])
                            elif j % 2 == 0:
                                nc.scalar.copy(dst, psf[:])
                            else:
                                nc.vector.tensor_copy(dst, psf[:])
                    r0 = bi * (s // 128) + 4 * qi
                    nc.sync.dma_start(out=out_d[:, r0:r0 + 4, :], in_=orow[:])

                # ---- software pipeline over batches; x prefetch distance 2
                issue_tr(0)
                issue_tr(1)
                chunks0 = make_proj_chunks(0)
                for c in chunks0[:6]:
                    c()
                carry = chunks0[6:]
                for bi in range(b):
                    if bi + 2 < b:
                        issue_tr(bi + 2)
                    if bi + 1 < b:
                        carry = carry + make_proj_chunks(bi + 1)
                    issue_attn(bi, carry)
                    carry = []
                    bt.pop(bi - 1, None)

            if repeat == 1:
                body()
            else:
                with tc.For_i(0, repeat, 1) as iv:
                    body(iv)

    nc.compile()
    return nc


_PROG = None


def _prep_in_maps(x, Wq, Wk, Wv, Wo, bq, bk):
    x = np.asarray(x, np.float32)
    b, s, e = x.shape
    # [p, b, st, ec, t] blocked layout, one contiguous 8KB run per (p, st)
    xx = x.reshape(b, s // QT_, QT_, e // 128, 128)
    xb = np.ascontiguousarray(xx.transpose(4, 0, 1, 3, 2)).astype(NBF)
    maps = []
    for c in range(NCORES):
        h0 = c * HPC
        def wcat(W):
            W = np.asarray(W, np.float32)
            return np.ascontiguousarray(
                np.concatenate([W[h0 + i] for i in range(HPC)], axis=1)
            ).astype(NBF)
        def bcat(bv_):
            bv_ = np.asarray(bv_, np.float32)
            return np.ascontiguousarray(
                np.concatenate([bv_[h0 + i] for i in range(HPC)])
            ).reshape(D2, 1).astype(np.float32)
        wo_sl = np.ascontiguousarray(
            np.asarray(Wo, np.float32)[h0 * D:(h0 + HPC) * D, :]).astype(NBF)
        maps.append({
            "xb": xb, "wq": wcat(Wq), "wk": wcat(Wk), "wv": wcat(Wv),
            "wo": wo_sl, "bq": bcat(bq), "bk": bcat(bk),
        })
    return maps


def kernel(x, Wq, bq, Wk, bk, Wv, bv, Wo, bo):
    global _PROG
    if _PROG is None:
        _PROG = build_program()
    x = np.asarray(x, np.float32)
    Wo = np.asarray(Wo, np.float32)
    maps = _prep_in_maps(x, Wq, Wk, Wv, Wo, bq, bk)
    res = run_bass_kernel_spmd(_PROG, maps, core_ids=list(range(NCORES)))
    acc = res.results[0]["out"].astype(np.float32)
    for c in range(1, NCORES):
        acc = acc + res.results[c]["out"]
    # unblock [p, r, e] -> row r*128+p
    acc = acc.reshape(128, B * S // 128, E).transpose(1, 0, 2).reshape(B * S, E)
    bias_vec = (np.asarray(bv, np.float32).reshape(-1) @ Wo
                + np.asarray(bo, np.float32))
    acc = acc + bias_vec[None, :]
    return acc.reshape(B, S, E)
